# revision 67
# baseline (speedup 1.0000x reference)
"""Trainium2 Bass kernel for DWT linear attention (nn_DWTLinearAttention).

Shards the 4 batch samples x 2 independent streams (x / y) across the 8
NeuronCores: core b handles x[b], core 4+b handles y[b].  Each core runs
the full per-sample pipeline in fp16 (the harness gate is rel_err < 2e-2;
fp16 end-to-end lands ~4e-4):

  FLAT (C=512, N=16384) fp16 view of the (N, C) input buffer, loaded once
  and held SBUF-resident (128 KB/partition).
  ll' = a+b+c+d  (2x2 haar low-pass, unscaled)                 (DVE)
  Q/K/V 1x1 convs from ll' with halved weights                 (PE fp16)
  row/col l2 normalization                                     (ACT+DVE)
  matrix' = [Kn;1]^T VT ; ksum ; tailor                        (PE+DVE)
  per 128-n' chunk jc:
    psP   = [Qn;1]^T @ matrix'                                 (PE)
    psL   = -0.25 * ll'^T      (eye-matmul transpose)          (PE)
    llt   = copy psL -> SBUF fp16                              (ACT)
    pscal = psP * (0.5*gamma*tailor) + llt                     (DVE stt)
    per 128-row out chunk wi:
      psO = x^T (4 eye-matmuls, fp16 rhs -> fp32 PSUM accum)
            + dup @ pscal      (partition-duplication matmul)  (PE)
      out_stage = copy psO -> fp16                             (ACT/DVE)
    one batched store DMA per jc (512 rows)

Output is written fp16 and upcast to fp32 on the host.
"""

import os
import sys

for _p in ("/opt/trn_rl_repo", "/root/.axon_site/_ro/trn_rl_repo"):
    if _p not in sys.path and os.path.isdir(_p):
        sys.path.append(_p)

import numpy as np

import concourse.bass as bass
import concourse.tile as tile
from concourse import bacc, mybir
from concourse import bass_utils

F16 = mybir.dt.float16
F32 = mybir.dt.float32
AF = mybir.ActivationFunctionType
ALU = mybir.AluOpType
ts = bass.ts

C = 512
N = 16384
NL = 4096        # low-band spatial size (64*64)
M = 64           # attention inner dim
EPS = 1e-6

# fp16 const blob column offsets; part1 [0:O_WV) holds everything the
# first attention chunks need, part2 the V weights, part3 phase-4/5 consts
O_ONES = 0        # 128
O_BKR = 128       # 64
O_BVB = 192       # 512
O_EYE = 704       # 128
O_NEYE = 832      # 128
O_DUPA = 960      # 128
O_DUPB = 1088     # 128
CB16_COLS = 1216
# fp8e4 weight blob offsets
O8_WQ = 0         # 4 * 64
O8_WK = 256       # 4 * 64
O8_WV = 512       # 4 * 512
CB8_COLS = 2560


def build_program():
    nc = bacc.Bacc(
        "TRN2",
        target_bir_lowering=False,
        debug=False,
        enable_asserts=True,
        num_devices=8,
    )

    d = {}
    d["xb"] = nc.dram_tensor("xb", [C, N], F16, kind="ExternalInput").ap()
    d["cb16"] = nc.dram_tensor("cb16", [128, CB16_COLS], F16,
                               kind="ExternalInput").ap()
    d["cb8"] = nc.dram_tensor("cb8", [128, CB8_COLS], mybir.dt.float8e4,
                              kind="ExternalInput").ap()
    d["cb32"] = nc.dram_tensor("cb32", [128, 66], F32,
                               kind="ExternalInput").ap()
    d["ones_row"] = nc.dram_tensor("ones_row", [1, NL], F16,
                                   kind="ExternalInput").ap()
    d["out"] = nc.dram_tensor("out", [N, C], F16, kind="ExternalOutput").ap()

    with tile.TileContext(nc) as tc:
        _emit(nc, tc, d)

    nc.compile()
    return nc


def _emit(nc, tc, d):
    from contextlib import ExitStack
    ctx = ExitStack()
    with ctx:
        ctx.enter_context(
            nc.allow_low_precision(reason="fp16 kernel; gate is 2e-2"))
        # ---------------- pools (PSUM: exactly 8 banks) ----------------
        ppM = ctx.enter_context(tc.tile_pool(name="ppM", bufs=1, space="PSUM"))
        ppKS = ctx.enter_context(tc.tile_pool(name="ppKS", bufs=1,
                                              space="PSUM"))
        ppA = ctx.enter_context(tc.tile_pool(name="ppA", bufs=3, space="PSUM"))
        ppB = ctx.enter_context(tc.tile_pool(name="ppB", bufs=2, space="PSUM"))
        ppC = ctx.enter_context(tc.tile_pool(name="ppC", bufs=1, space="PSUM"))

        cpool = ctx.enter_context(tc.tile_pool(name="consts", bufs=1))
        xpool = ctx.enter_context(tc.tile_pool(name="xres", bufs=1))
        llpool = ctx.enter_context(tc.tile_pool(name="ll", bufs=1))
        qnpool = ctx.enter_context(tc.tile_pool(name="qn", bufs=1))
        vpool = ctx.enter_context(tc.tile_pool(name="vtmp", bufs=2))
        sqpool = ctx.enter_context(tc.tile_pool(name="sq", bufs=1))
        nrmpool = ctx.enter_context(tc.tile_pool(name="nrm", bufs=1))
        kpool = ctx.enter_context(tc.tile_pool(name="kpre", bufs=2))
        ktpool = ctx.enter_context(tc.tile_pool(name="knt", bufs=3))
        vtpool = ctx.enter_context(tc.tile_pool(name="vt", bufs=3))
        stpool = ctx.enter_context(tc.tile_pool(name="stat", bufs=4))
        mspool = ctx.enter_context(tc.tile_pool(name="ms", bufs=1))
        pspool = ctx.enter_context(tc.tile_pool(name="pscal", bufs=3))
        ltpool = ctx.enter_context(tc.tile_pool(name="llt", bufs=2))
        stagepool = ctx.enter_context(tc.tile_pool(name="stage", bufs=3))

        # ---------------- constants ----------------
        # cb32 first (tiny, needed by kpre); cb16 split into three DMAs so
        # x loads for the first strip aren't stuck behind the blob.
        cb32 = cpool.tile([128, 66], F32, tag="c32")
        cb16 = cpool.tile([128, CB16_COLS], F16, tag="c16")
        cb8 = cpool.tile([128, CB8_COLS], mybir.dt.float8e4, tag="c8")

        def wq_cb(cb):
            return cb8[:, O8_WQ + cb * 64:O8_WQ + (cb + 1) * 64]

        def wk_cb(cb):
            return cb8[:, O8_WK + cb * 64:O8_WK + (cb + 1) * 64]

        def wv_cb(cb):
            return cb8[:, O8_WV + cb * 512:O8_WV + (cb + 1) * 512]

        bvb = cb16[:, O_BVB:O_BVB + 512]
        bkr = cb16[:, O_BKR:O_BKR + 64]
        eye = cb16[:, O_EYE:O_EYE + 128]
        neye = cb16[:, O_NEYE:O_NEYE + 128]
        dupA = cb16[:, O_DUPA:O_DUPA + 128]
        dupB = cb16[:, O_DUPB:O_DUPB + 128]
        ones = cb16[:, O_ONES:O_ONES + 128]
        bq = cb32[0:M, 0:1]
        g2 = cb32[:, 1:2]

        x4 = xpool.tile([128, 4, N], F16, tag="x4")
        ll4 = llpool.tile([128, 4, NL], F16, tag="ll4")
        qn = qnpool.tile([M + 1, NL], F16, tag="qn")

        psM = ppM.tile([M + 1, 512], F32, tag="m", name="psM")
        psKS = ppKS.tile([M, 1], F32, tag="ks", name="psKS")

        # ------- phase 1: load strip + haar low-pass -------
        def p1_load(cb, c0, cols, eng=None):
            (eng or nc.sync).dma_start(
                x4[:, cb, c0:c0 + cols],
                d["xb"][ts(cb, 128), c0:c0 + cols])

        def p1_dwt(cb, c0, cols, jeng):
            xs = x4[:, cb, c0:c0 + cols].rearrange(
                "p (i t j) -> p i t j", t=2, j=128)
            nh = cols // 2
            v = vpool.tile([128, 1024], F16, tag="v", name="v")
            # row-pair sums first: packed inner dim -> DVE 2x mode
            nc.vector.tensor_add(
                v[0:128, 0:nh].rearrange("p (i j) -> p i j", j=128),
                xs[:, :, 0:1, :], xs[:, :, 1:2, :])
            vv = v[0:128, 0:nh].rearrange("p (i k t) -> p i k t", t=2, k=64)
            llv = ll4[:, cb, c0 // 4:c0 // 4 + cols // 4].rearrange(
                "p (i k) -> p i k", k=64)
            if jeng == "pool":
                nc.gpsimd.tensor_add(llv, vv[:, :, :, 0:1],
                                     vv[:, :, :, 1:2])
            else:
                nc.vector.tensor_add(llv, vv[:, :, :, 0:1],
                                     vv[:, :, :, 1:2])

        # ------- phase 3 pieces (software-pipelined) -------
        kv_state = {}

        def p3_mm(kc):
            # PE front half: K matmuls (+ rank-1 bk bias) and V matmuls.
            # The V bias is folded into matrix' later as the rank-1 term
            # ksum * bv^T, so vt is a plain PSUM->SBUF copy.
            psK = ppB.tile([128, M], F32, tag="b", name="psK")
            for cb in range(4):
                nc.tensor.matmul(
                    psK[:], ll4[:, cb, ts(kc, 128)], wk_cb(cb),
                    start=(cb == 0), stop=False)
            nc.tensor.matmul(psK[:], ones[0:1, :], bkr[0:1, :],
                             start=False, stop=True)
            psV = ppA.tile([128, 512], F32, tag="a", name="psV")
            for cb in range(4):
                nc.tensor.matmul(
                    psV[:], ll4[:, cb, ts(kc, 128)], wv_cb(cb),
                    start=(cb == 0), stop=(cb == 3))
            # K row normalization (GPSIMD may not touch PSUM: DVE/ACT only).
            # kpre copy releases the psK bank quickly; the rest runs from
            # SBUF fp16 where DVE gets its fast modes.
            kpre = kpool.tile([128, M], F16, tag="kp", name="kpre")
            nc.scalar.copy(kpre[:], psK[:])
            scr = kpool.tile([128, M], F16, tag="scr", name="scr")
            ssq = stpool.tile([128, 1], F32, tag="ssq", name="ssq")
            nc.vector.scalar_tensor_tensor(scr[:], kpre[:], 1.0, kpre[:],
                                           op0=ALU.mult, op1=ALU.mult,
                                           accum_out=ssq[:])
            nrm2 = stpool.tile([128, 1], F32, tag="nrm2", name="nrm2")
            nc.scalar.sqrt(nrm2[:], ssq[:])
            ik = stpool.tile([128, 1], F32, tag="ik", name="ik")
            nc.vector.reciprocal(ik[:], nrm2[:])
            knt = ktpool.tile([128, M + 1], F16, tag="knt", name="knt")
            nc.vector.tensor_scalar_mul(knt[:, 0:M], kpre[:], ik[:, 0:1])
            nc.vector.memset(knt[:, M:M + 1], 1.0)
            vt = vtpool.tile([128, 512], F16, tag="vt", name="vt")
            if kc % 4 == 0:
                nc.vector.tensor_copy(vt[:], psV[:])
            else:
                nc.scalar.copy(vt[:], psV[:])
            kv_state[kc] = (knt, vt)

        def p3_acc(kc):
            # PE back half: accumulate matrix'/ksum (one kc behind).
            # psM's group is closed later by the rank-1 V-bias term.
            knt, vt = kv_state.pop(kc)
            nc.tensor.matmul(psM[:], knt[:], vt[:],
                             start=(kc == 0), stop=False,
                             skip_group_check=True)
            nc.tensor.matmul(psKS[:], knt[:, 0:M], ones[:, 0:1],
                             start=(kc == 0), stop=(kc == 31))

        q_state = {}

        def p2_a(qc):
            psQ = ppA.tile([M, 512], F32, tag="a", name="psQ")
            for cb in range(4):
                nc.tensor.matmul(
                    psQ[:], wq_cb(cb), ll4[:, cb, ts(qc, 512)],
                    start=(cb == 0), stop=(cb == 3))
            sq = sqpool.tile([M, 512], F16, tag="sq", name="sq")
            nc.scalar.activation(sq[:], psQ[:], AF.Square,
                                 bias=bq, scale=1.0)
            q_state[qc] = (psQ, sq)

        def p2_b(qc):
            psQ, sq = q_state[qc]
            psSS = ppC.tile([128, 512], F32, tag="c", name="psSS")
            nc.tensor.matmul(psSS[:], ones[0:M, :], sq[:],
                             start=True, stop=True)
            nrm = nrmpool.tile([1, 512], F16, tag="nrm", name="nrm")
            nc.scalar.sqrt(nrm[:], psSS[0:1, :])
            inv = nrmpool.tile([1, 512], F16, tag="inv", name="inv")
            nc.vector.reciprocal(inv[:], nrm[:])
            q_state[qc] = (psQ, inv)

        def p2_c(qc):
            psQ, inv = q_state.pop(qc)
            psB = ppB.tile([128, 512], F32, tag="b", name="psB")
            nc.tensor.matmul(psB[:], ones[0:1, :], inv[:],
                             start=True, stop=True)
            bcs = sqpool.tile([M, 512], F16, tag="bcs", name="bcs")
            nc.scalar.copy(bcs[:], psB[0:M, :])
            nc.vector.scalar_tensor_tensor(
                qn[0:M, ts(qc, 512)], psQ[:], bq[:, 0:1], bcs[:],
                op0=ALU.add, op1=ALU.mult)

        # ------- interleaved phases 1+2+3 -------
        # startup: first strip in halves, const blob split around them
        for cb in range(4):
            p1_load(cb, 0, 1024)
        nc.sync.dma_start(cb32[:], d["cb32"])
        nc.sync.dma_start(cb16[:, 0:O_EYE], d["cb16"][:, 0:O_EYE])
        nc.sync.dma_start(cb8[:, 0:512], d["cb8"][:, 0:512])
        for cb in range(4):
            p1_dwt(cb, 0, 1024, "dve")
        nc.sync.dma_start(cb8[:, 512:CB8_COLS], d["cb8"][:, 512:CB8_COLS])
        for cb in range(4):
            p1_load(cb, 1024, 1024)
        for cb in range(4):
            p1_dwt(cb, 1024, 1024, "dve")
        nc.sync.dma_start(cb16[:, O_EYE:CB16_COLS],
                          d["cb16"][:, O_EYE:CB16_COLS])
        nc.sync.dma_start(qn[M:M + 1, :], d["ones_row"])
        for ws in range(1, 8):
            for cb in range(4):
                p1_load(cb, ws * 2048, 2048)
                p1_dwt(cb, ws * 2048, 2048, "pool")

        ksum = mspool.tile([M + 1, 1], F16, tag="ksum")
        nc.vector.memset(ksum[M:M + 1, :], float(NL))

        for kc in range(32):
            grp, ph = divmod(kc, 4)
            p3_mm(kc)
            if ph == 1:
                p2_a(grp)
            elif ph == 2:
                p2_b(grp)
            elif ph == 3:
                p2_c(grp)
            if kc > 1:
                p3_acc(kc - 2)
        p3_acc(30)
        p3_acc(31)

        # ------- phase 3.5: matrix / ksum tail / tailor -------
        # psL(0) keeps PE busy while ACT/DVE produce matrix / tailor
        def p4_psL(jc):
            psL = ppB.tile([128, 512], F32, tag="b", name="psL")
            for cb in range(4):
                nc.tensor.matmul(psL[:, ts(cb, 128)],
                                 ll4[:, cb, ts(jc, 128)], neye,
                                 start=(cb == 0), stop=(cb == 3),
                                 skip_group_check=True)
            return psL

        psL0 = p4_psL(0)
        nc.vector.tensor_scalar_add(ksum[0:M, :], psKS[:], EPS)
        # rank-1 V-bias: matrix' += ksum * bv^T  (ksum row transposed on PE)
        psKr = ppB.tile([1, M + 1], F32, tag="b", name="psKr")
        nc.tensor.matmul(psKr[:], ksum[:], eye[0:M + 1, 0:M + 1],
                         start=True, stop=True)
        ksrow = mspool.tile([1, M + 1], F16, tag="ksr")
        nc.vector.tensor_copy(ksrow[:], psKr[:])
        nc.tensor.matmul(psM[:], ksrow[:], bvb[0:1, :],
                         start=False, stop=True, skip_group_check=True)
        matrix = mspool.tile([M + 1, 512], F16, tag="mx")
        nc.scalar.copy(matrix[:], psM[:])
        psT = ppKS.tile([128, 32], F32, tag="ks", name="psT")
        for jc in range(32):
            nc.tensor.matmul(psT[:, jc:jc + 1], qn[:, ts(jc, 128)],
                             ksum[:], start=True, stop=True,
                             skip_group_check=True)
        sT = mspool.tile([128, 32], F32, tag="sT")
        nc.vector.reciprocal(sT[:], psT[:])
        sTg = mspool.tile([128, 32], F32, tag="sTg")
        nc.vector.tensor_scalar_mul(sTg[:], sT[:], g2[:, 0:1])

        # ------- phases 4+5 (psP/psL/pscal pipelined one jc ahead) -------
        def p4_head(jc, psL=None):
            psP = ppB.tile([128, 512], F32, tag="b", name="psP")
            nc.tensor.matmul(psP[:], qn[:, ts(jc, 128)], matrix[:],
                             start=True, stop=True)
            if psL is None:
                psL = p4_psL(jc)
            # DVE may read only one PSUM operand: stage psL through SBUF
            llt = ltpool.tile([128, 512], F16, tag="lt", name="llt")
            nc.scalar.copy(llt[:], psL[:])
            pscal = pspool.tile([128, 512], F16, tag="ps", name="pscal")
            nc.vector.scalar_tensor_tensor(
                pscal[:], psP[:], sTg[:, jc:jc + 1], llt[:],
                op0=ALU.mult, op1=ALU.add)
            return pscal

        def pso_mm(jc, wi, close=False):
            w = 4 * jc + wi
            psO = ppA.tile([128, 512], F32, tag="a", name="psO")
            for cb in range(4):
                nc.tensor.matmul(psO[:, ts(cb, 128)],
                                 x4[:, cb, w * 128:(w + 1) * 128],
                                 eye, start=(cb == 0),
                                 stop=(close and cb == 3),
                                 skip_group_check=True)
            return psO

        def pso_fin(psO, pscal, stage, wi):
            nc.tensor.matmul(psO[:], dupA if wi < 2 else dupB,
                             pscal[:], start=False, stop=True,
                             skip_group_check=True)
            dst = stage[:, wi, :]
            if wi % 2 == 1:
                nc.vector.tensor_copy(dst, psO[:])
            else:
                nc.scalar.copy(dst, psO[:])

        hoist = [pso_mm(0, 0), pso_mm(0, 1)]
        pq = [p4_head(0, psL=psL0), p4_head(1)]
        for jc in range(32):
            pscal = pq.pop(0)
            stage = stagepool.tile([128, 4, 512], F16, tag="st",
                                   name="stage")
            if jc == 0:
                ps0, ps1 = hoist
            else:
                ps0 = pso_mm(jc, 0)
                ps1 = pso_mm(jc, 1)
            pso_fin(ps0, pscal, stage, 0)
            if jc < 30:
                pq.append(p4_head(jc + 2))
            pso_fin(ps1, pscal, stage, 1)
            ps2 = pso_mm(jc, 2)
            pso_fin(ps2, pscal, stage, 2)
            ps3 = pso_mm(jc, 3)
            pso_fin(ps3, pscal, stage, 3)
            dview = d["out"][jc * 512:(jc + 1) * 512, :].rearrange(
                "(wi p) c -> p wi c", p=128)
            if jc < 31:
                nc.sync.dma_start(dview, stage[:])
            else:
                # split the last store so the tail drains sooner
                nc.sync.dma_start(dview[:, 0:2, :], stage[:, 0:2, :])
                nc.sync.dma_start(dview[:, 2:3, :], stage[:, 2:3, :])
                nc.sync.dma_start(dview[:, 3:4, :], stage[:, 3:4, :])


# ------------------------------------------------------------------
# host-side wrapper
# ------------------------------------------------------------------
_NC_CACHE = None


def _get_program():
    global _NC_CACHE
    if _NC_CACHE is None:
        _NC_CACHE = build_program()
    return _NC_CACHE


def _make_in_map(xb, wq, bq, wk, bk, wv, bv, gamma):
    g = float(np.asarray(gamma).reshape(-1)[0])

    import ml_dtypes
    f8 = ml_dtypes.float8_e4m3
    cb16 = np.zeros((128, CB16_COLS), dtype=np.float16)
    cb8 = np.zeros((128, CB8_COLS), dtype=f8)
    wqT = (0.5 * np.asarray(wq, np.float32)).T    # (C, M)
    wkT = (0.5 * np.asarray(wk, np.float32)).T
    wvT = (0.5 * np.asarray(wv, np.float32)).T    # (C, C)
    for cb in range(4):
        rows = slice(cb * 128, (cb + 1) * 128)
        cb8[:, O8_WQ + cb * 64:O8_WQ + (cb + 1) * 64] = wqT[rows].astype(f8)
        cb8[:, O8_WK + cb * 64:O8_WK + (cb + 1) * 64] = wkT[rows].astype(f8)
        cb8[:, O8_WV + cb * 512:O8_WV + (cb + 1) * 512] = wvT[rows].astype(f8)
    cb16[:, O_BVB:O_BVB + 512] = np.asarray(bv, np.float32)[None, :]
    cb16[:, O_BKR:O_BKR + 64] = np.asarray(bk, np.float32)[None, :]
    ey = np.eye(128, dtype=np.float16)
    cb16[:, O_EYE:O_EYE + 128] = ey
    cb16[:, O_NEYE:O_NEYE + 128] = -0.25 * ey
    r = np.arange(128)
    dupA = np.zeros((128, 128), dtype=np.float16)
    dupA[r // 2, r] = 1.0
    dupB = np.zeros((128, 128), dtype=np.float16)
    dupB[64 + r // 2, r] = 1.0
    cb16[:, O_DUPA:O_DUPA + 128] = dupA
    cb16[:, O_DUPB:O_DUPB + 128] = dupB
    cb16[:, O_ONES:O_ONES + 128] = 1.0

    cb32 = np.zeros((128, 66), dtype=np.float32)
    cb32[0:M, 0] = np.asarray(bq, np.float32)
    cb32[:, 1] = 0.5 * g
    cb32[:, 2:66] = np.asarray(bk, np.float32)[None, :]

    return {
        "xb": np.ascontiguousarray(
            np.asarray(xb).reshape(C, N)).astype(np.float16),
        "cb16": cb16,
        "cb8": cb8,
        "cb32": cb32,
        "ones_row": np.ones((1, NL), dtype=np.float16),
    }


def kernel(x, y, gamma, gamma_y, wq, bq, wk, bk, wv, bv,
           wqy, bqy, wky, bky, wvy, bvy):
    x = np.asarray(x, dtype=np.float32)
    y = np.asarray(y, dtype=np.float32)
    B = x.shape[0]
    assert x.shape == (B, N, C), x.shape

    nc = _get_program()
    in_maps = []
    for b in range(B):
        in_maps.append(_make_in_map(x[b], wq, bq, wk, bk, wv, bv, gamma))
    for b in range(B):
        in_maps.append(_make_in_map(y[b], wqy, bqy, wky, bky, wvy, bvy,
                                    gamma_y))
    res = bass_utils.run_bass_kernel_spmd(
        nc, in_maps, core_ids=list(range(8)))
    out_x = np.stack([res.results[b]["out"].astype(np.float32)
                      for b in range(B)])
    out_y = np.stack([res.results[B + b]["out"].astype(np.float32)
                      for b in range(B)])
    return (out_x, out_y)


# revision 68
# speedup vs baseline: 1.0055x; 1.0055x over previous
"""Trainium2 Bass kernel for DWT linear attention (nn_DWTLinearAttention).

Shards the 4 batch samples x 2 independent streams (x / y) across the 8
NeuronCores: core b handles x[b], core 4+b handles y[b].  Each core runs
the full per-sample pipeline in fp16 (the harness gate is rel_err < 2e-2;
fp16 end-to-end lands ~4e-4):

  FLAT (C=512, N=16384) fp16 view of the (N, C) input buffer, loaded once
  and held SBUF-resident (128 KB/partition).
  ll' = a+b+c+d  (2x2 haar low-pass, unscaled)                 (DVE)
  Q/K/V 1x1 convs from ll' with halved weights                 (PE fp16)
  row/col l2 normalization                                     (ACT+DVE)
  matrix' = [Kn;1]^T VT ; ksum ; tailor                        (PE+DVE)
  per 128-n' chunk jc:
    psP   = [Qn;1]^T @ matrix'                                 (PE)
    psL   = -0.25 * ll'^T      (eye-matmul transpose)          (PE)
    llt   = copy psL -> SBUF fp16                              (ACT)
    pscal = psP * (0.5*gamma*tailor) + llt                     (DVE stt)
    per 128-row out chunk wi:
      psO = x^T (4 eye-matmuls, fp16 rhs -> fp32 PSUM accum)
            + dup @ pscal      (partition-duplication matmul)  (PE)
      out_stage = copy psO -> fp16                             (ACT/DVE)
    one batched store DMA per jc (512 rows)

Output is written fp16 and upcast to fp32 on the host.
"""

import os
import sys

for _p in ("/opt/trn_rl_repo", "/root/.axon_site/_ro/trn_rl_repo"):
    if _p not in sys.path and os.path.isdir(_p):
        sys.path.append(_p)

import numpy as np

import concourse.bass as bass
import concourse.tile as tile
from concourse import bacc, mybir
from concourse import bass_utils

F16 = mybir.dt.float16
F32 = mybir.dt.float32
AF = mybir.ActivationFunctionType
ALU = mybir.AluOpType
ts = bass.ts

C = 512
N = 16384
NL = 4096        # low-band spatial size (64*64)
M = 64           # attention inner dim
EPS = 1e-6

# fp16 const blob column offsets; part1 [0:O_WV) holds everything the
# first attention chunks need, part2 the V weights, part3 phase-4/5 consts
O_ONES = 0        # 128
O_BKR = 128       # 64
O_BVB = 192       # 512
O_EYE = 704       # 128
O_NEYE = 832      # 128
O_DUPA = 960      # 128
O_DUPB = 1088     # 128
CB16_COLS = 1216
# fp8e4 weight blob offsets
O8_WQ = 0         # 4 * 64
O8_WK = 256       # 4 * 64
O8_WV = 512       # 4 * 512
CB8_COLS = 2560


def build_program():
    nc = bacc.Bacc(
        "TRN2",
        target_bir_lowering=False,
        debug=False,
        enable_asserts=True,
        num_devices=8,
    )

    d = {}
    d["xb"] = nc.dram_tensor("xb", [C, N], F16, kind="ExternalInput").ap()
    d["cb16"] = nc.dram_tensor("cb16", [128, CB16_COLS], F16,
                               kind="ExternalInput").ap()
    d["cb8"] = nc.dram_tensor("cb8", [128, CB8_COLS], mybir.dt.float8e4,
                              kind="ExternalInput").ap()
    d["cb32"] = nc.dram_tensor("cb32", [128, 66], F32,
                               kind="ExternalInput").ap()
    d["ones_row"] = nc.dram_tensor("ones_row", [1, NL], F16,
                                   kind="ExternalInput").ap()
    d["out"] = nc.dram_tensor("out", [N, C], F16, kind="ExternalOutput").ap()

    with tile.TileContext(nc) as tc:
        _emit(nc, tc, d)

    nc.compile()
    return nc


def _emit(nc, tc, d):
    from contextlib import ExitStack
    ctx = ExitStack()
    with ctx:
        ctx.enter_context(
            nc.allow_low_precision(reason="fp16 kernel; gate is 2e-2"))
        # ---------------- pools (PSUM: exactly 8 banks) ----------------
        ppM = ctx.enter_context(tc.tile_pool(name="ppM", bufs=1, space="PSUM"))
        ppKS = ctx.enter_context(tc.tile_pool(name="ppKS", bufs=1,
                                              space="PSUM"))
        ppA = ctx.enter_context(tc.tile_pool(name="ppA", bufs=3, space="PSUM"))
        ppB = ctx.enter_context(tc.tile_pool(name="ppB", bufs=2, space="PSUM"))
        ppC = ctx.enter_context(tc.tile_pool(name="ppC", bufs=1, space="PSUM"))

        cpool = ctx.enter_context(tc.tile_pool(name="consts", bufs=1))
        xpool = ctx.enter_context(tc.tile_pool(name="xres", bufs=1))
        llpool = ctx.enter_context(tc.tile_pool(name="ll", bufs=1))
        qnpool = ctx.enter_context(tc.tile_pool(name="qn", bufs=1))
        vpool = ctx.enter_context(tc.tile_pool(name="vtmp", bufs=2))
        sqpool = ctx.enter_context(tc.tile_pool(name="sq", bufs=1))
        nrmpool = ctx.enter_context(tc.tile_pool(name="nrm", bufs=1))
        kpool = ctx.enter_context(tc.tile_pool(name="kpre", bufs=2))
        ktpool = ctx.enter_context(tc.tile_pool(name="knt", bufs=3))
        vtpool = ctx.enter_context(tc.tile_pool(name="vt", bufs=3))
        stpool = ctx.enter_context(tc.tile_pool(name="stat", bufs=4))
        mspool = ctx.enter_context(tc.tile_pool(name="ms", bufs=1))
        pspool = ctx.enter_context(tc.tile_pool(name="pscal", bufs=3))
        ltpool = ctx.enter_context(tc.tile_pool(name="llt", bufs=2))
        stagepool = ctx.enter_context(tc.tile_pool(name="stage", bufs=5))

        # ---------------- constants ----------------
        # cb32 first (tiny, needed by kpre); cb16 split into three DMAs so
        # x loads for the first strip aren't stuck behind the blob.
        cb32 = cpool.tile([128, 66], F32, tag="c32")
        cb16 = cpool.tile([128, CB16_COLS], F16, tag="c16")
        cb8 = cpool.tile([128, CB8_COLS], mybir.dt.float8e4, tag="c8")

        def wq_cb(cb):
            return cb8[:, O8_WQ + cb * 64:O8_WQ + (cb + 1) * 64]

        def wk_cb(cb):
            return cb8[:, O8_WK + cb * 64:O8_WK + (cb + 1) * 64]

        def wv_cb(cb):
            return cb8[:, O8_WV + cb * 512:O8_WV + (cb + 1) * 512]

        bvb = cb16[:, O_BVB:O_BVB + 512]
        bkr = cb16[:, O_BKR:O_BKR + 64]
        eye = cb16[:, O_EYE:O_EYE + 128]
        neye = cb16[:, O_NEYE:O_NEYE + 128]
        dupA = cb16[:, O_DUPA:O_DUPA + 128]
        dupB = cb16[:, O_DUPB:O_DUPB + 128]
        ones = cb16[:, O_ONES:O_ONES + 128]
        bq = cb32[0:M, 0:1]
        g2 = cb32[:, 1:2]

        x4 = xpool.tile([128, 4, N], F16, tag="x4")
        ll4 = llpool.tile([128, 4, NL], F16, tag="ll4")
        qn = qnpool.tile([M + 1, NL], F16, tag="qn")

        psM = ppM.tile([M + 1, 512], F32, tag="m", name="psM")
        psKS = ppKS.tile([M, 1], F32, tag="ks", name="psKS")

        # ------- phase 1: load strip + haar low-pass -------
        def p1_load(cb, c0, cols, eng=None):
            (eng or nc.sync).dma_start(
                x4[:, cb, c0:c0 + cols],
                d["xb"][ts(cb, 128), c0:c0 + cols])

        def p1_dwt(cb, c0, cols, jeng):
            xs = x4[:, cb, c0:c0 + cols].rearrange(
                "p (i t j) -> p i t j", t=2, j=128)
            nh = cols // 2
            v = vpool.tile([128, 1024], F16, tag="v", name="v")
            # row-pair sums first: packed inner dim -> DVE 2x mode
            nc.vector.tensor_add(
                v[0:128, 0:nh].rearrange("p (i j) -> p i j", j=128),
                xs[:, :, 0:1, :], xs[:, :, 1:2, :])
            vv = v[0:128, 0:nh].rearrange("p (i k t) -> p i k t", t=2, k=64)
            llv = ll4[:, cb, c0 // 4:c0 // 4 + cols // 4].rearrange(
                "p (i k) -> p i k", k=64)
            if jeng == "pool":
                nc.gpsimd.tensor_add(llv, vv[:, :, :, 0:1],
                                     vv[:, :, :, 1:2])
            else:
                nc.vector.tensor_add(llv, vv[:, :, :, 0:1],
                                     vv[:, :, :, 1:2])

        # ------- phase 3 pieces (software-pipelined) -------
        kv_state = {}

        def p3_mm(kc):
            # PE front half: K matmuls (+ rank-1 bk bias) and V matmuls.
            # The V bias is folded into matrix' later as the rank-1 term
            # ksum * bv^T, so vt is a plain PSUM->SBUF copy.
            psK = ppB.tile([128, M], F32, tag="b", name="psK")
            for cb in range(4):
                nc.tensor.matmul(
                    psK[:], ll4[:, cb, ts(kc, 128)], wk_cb(cb),
                    start=(cb == 0), stop=False)
            nc.tensor.matmul(psK[:], ones[0:1, :], bkr[0:1, :],
                             start=False, stop=True)
            psV = ppA.tile([128, 512], F32, tag="a", name="psV")
            for cb in range(4):
                nc.tensor.matmul(
                    psV[:], ll4[:, cb, ts(kc, 128)], wv_cb(cb),
                    start=(cb == 0), stop=(cb == 3))
            # K row normalization (GPSIMD may not touch PSUM: DVE/ACT only).
            # kpre copy releases the psK bank quickly; the rest runs from
            # SBUF fp16 where DVE gets its fast modes.
            kpre = kpool.tile([128, M], F16, tag="kp", name="kpre")
            nc.scalar.copy(kpre[:], psK[:])
            scr = kpool.tile([128, M], F16, tag="scr", name="scr")
            ssq = stpool.tile([128, 1], F32, tag="ssq", name="ssq")
            nc.vector.scalar_tensor_tensor(scr[:], kpre[:], 1.0, kpre[:],
                                           op0=ALU.mult, op1=ALU.mult,
                                           accum_out=ssq[:])
            nrm2 = stpool.tile([128, 1], F32, tag="nrm2", name="nrm2")
            nc.scalar.sqrt(nrm2[:], ssq[:])
            ik = stpool.tile([128, 1], F32, tag="ik", name="ik")
            nc.vector.reciprocal(ik[:], nrm2[:])
            knt = ktpool.tile([128, M + 1], F16, tag="knt", name="knt")
            nc.vector.tensor_scalar_mul(knt[:, 0:M], kpre[:], ik[:, 0:1])
            nc.vector.memset(knt[:, M:M + 1], 1.0)
            vt = vtpool.tile([128, 512], F16, tag="vt", name="vt")
            if kc % 4 == 0:
                nc.vector.tensor_copy(vt[:], psV[:])
            else:
                nc.scalar.copy(vt[:], psV[:])
            kv_state[kc] = (knt, vt)

        def p3_acc(kc):
            # PE back half: accumulate matrix'/ksum (one kc behind).
            # psM's group is closed later by the rank-1 V-bias term.
            knt, vt = kv_state.pop(kc)
            nc.tensor.matmul(psM[:], knt[:], vt[:],
                             start=(kc == 0), stop=False,
                             skip_group_check=True)
            nc.tensor.matmul(psKS[:], knt[:, 0:M], ones[:, 0:1],
                             start=(kc == 0), stop=(kc == 31))

        q_state = {}

        def p2_a(qc):
            psQ = ppA.tile([M, 512], F32, tag="a", name="psQ")
            for cb in range(4):
                nc.tensor.matmul(
                    psQ[:], wq_cb(cb), ll4[:, cb, ts(qc, 512)],
                    start=(cb == 0), stop=(cb == 3))
            sq = sqpool.tile([M, 512], F16, tag="sq", name="sq")
            nc.scalar.activation(sq[:], psQ[:], AF.Square,
                                 bias=bq, scale=1.0)
            q_state[qc] = (psQ, sq)

        def p2_b(qc):
            psQ, sq = q_state[qc]
            psSS = ppC.tile([128, 512], F32, tag="c", name="psSS")
            nc.tensor.matmul(psSS[:], ones[0:M, :], sq[:],
                             start=True, stop=True)
            nrm = nrmpool.tile([1, 512], F16, tag="nrm", name="nrm")
            nc.scalar.sqrt(nrm[:], psSS[0:1, :])
            inv = nrmpool.tile([1, 512], F16, tag="inv", name="inv")
            nc.vector.reciprocal(inv[:], nrm[:])
            q_state[qc] = (psQ, inv)

        def p2_c(qc):
            psQ, inv = q_state.pop(qc)
            psB = ppB.tile([128, 512], F32, tag="b", name="psB")
            nc.tensor.matmul(psB[:], ones[0:1, :], inv[:],
                             start=True, stop=True)
            bcs = sqpool.tile([M, 512], F16, tag="bcs", name="bcs")
            nc.scalar.copy(bcs[:], psB[0:M, :])
            nc.vector.scalar_tensor_tensor(
                qn[0:M, ts(qc, 512)], psQ[:], bq[:, 0:1], bcs[:],
                op0=ALU.add, op1=ALU.mult)

        # ------- interleaved phases 1+2+3 -------
        # startup: first strip in halves, const blob split around them
        for cb in range(4):
            p1_load(cb, 0, 1024)
        nc.sync.dma_start(cb32[:], d["cb32"])
        nc.sync.dma_start(cb16[:, 0:O_EYE], d["cb16"][:, 0:O_EYE])
        nc.sync.dma_start(cb8[:, 0:512], d["cb8"][:, 0:512])
        for cb in range(4):
            p1_dwt(cb, 0, 1024, "dve")
        nc.sync.dma_start(cb8[:, 512:CB8_COLS], d["cb8"][:, 512:CB8_COLS])
        for cb in range(4):
            p1_load(cb, 1024, 1024)
        for cb in range(4):
            p1_dwt(cb, 1024, 1024, "dve")
        nc.sync.dma_start(cb16[:, O_EYE:CB16_COLS],
                          d["cb16"][:, O_EYE:CB16_COLS])
        nc.sync.dma_start(qn[M:M + 1, :], d["ones_row"])
        for ws in range(1, 8):
            for cb in range(4):
                p1_load(cb, ws * 2048, 2048)
                p1_dwt(cb, ws * 2048, 2048, "pool")

        ksum = mspool.tile([M + 1, 1], F16, tag="ksum")
        nc.vector.memset(ksum[M:M + 1, :], float(NL))

        for kc in range(32):
            grp, ph = divmod(kc, 4)
            p3_mm(kc)
            if ph == 1:
                p2_a(grp)
            elif ph == 2:
                p2_b(grp)
            elif ph == 3:
                p2_c(grp)
            if kc > 1:
                p3_acc(kc - 2)
        p3_acc(30)
        p3_acc(31)

        # ------- phase 3.5: matrix / ksum tail / tailor -------
        # psL(0) keeps PE busy while ACT/DVE produce matrix / tailor
        def p4_psL(jc):
            psL = ppB.tile([128, 512], F32, tag="b", name="psL")
            for cb in range(4):
                nc.tensor.matmul(psL[:, ts(cb, 128)],
                                 ll4[:, cb, ts(jc, 128)], neye,
                                 start=(cb == 0), stop=(cb == 3),
                                 skip_group_check=True)
            return psL

        psL0 = p4_psL(0)
        nc.vector.tensor_scalar_add(ksum[0:M, :], psKS[:], EPS)
        # rank-1 V-bias: matrix' += ksum * bv^T  (ksum row transposed on PE)
        psKr = ppB.tile([1, M + 1], F32, tag="b", name="psKr")
        nc.tensor.matmul(psKr[:], ksum[:], eye[0:M + 1, 0:M + 1],
                         start=True, stop=True)
        ksrow = mspool.tile([1, M + 1], F16, tag="ksr")
        nc.vector.tensor_copy(ksrow[:], psKr[:])
        nc.tensor.matmul(psM[:], ksrow[:], bvb[0:1, :],
                         start=False, stop=True, skip_group_check=True)
        matrix = mspool.tile([M + 1, 512], F16, tag="mx")
        nc.scalar.copy(matrix[:], psM[:])
        psT = ppKS.tile([128, 32], F32, tag="ks", name="psT")
        for jc in range(32):
            nc.tensor.matmul(psT[:, jc:jc + 1], qn[:, ts(jc, 128)],
                             ksum[:], start=True, stop=True,
                             skip_group_check=True)
        sT = mspool.tile([128, 32], F32, tag="sT")
        nc.vector.reciprocal(sT[:], psT[:])
        sTg = mspool.tile([128, 32], F32, tag="sTg")
        nc.vector.tensor_scalar_mul(sTg[:], sT[:], g2[:, 0:1])

        # ------- phases 4+5 (psP/psL/pscal pipelined one jc ahead) -------
        def p4_head(jc, psL=None):
            psP = ppB.tile([128, 512], F32, tag="b", name="psP")
            nc.tensor.matmul(psP[:], qn[:, ts(jc, 128)], matrix[:],
                             start=True, stop=True)
            if psL is None:
                psL = p4_psL(jc)
            # DVE may read only one PSUM operand: stage psL through SBUF
            llt = ltpool.tile([128, 512], F16, tag="lt", name="llt")
            nc.scalar.copy(llt[:], psL[:])
            pscal = pspool.tile([128, 512], F16, tag="ps", name="pscal")
            nc.vector.scalar_tensor_tensor(
                pscal[:], psP[:], sTg[:, jc:jc + 1], llt[:],
                op0=ALU.mult, op1=ALU.add)
            return pscal

        def pso_mm(jc, wi, close=False):
            w = 4 * jc + wi
            psO = ppA.tile([128, 512], F32, tag="a", name="psO")
            for cb in range(4):
                nc.tensor.matmul(psO[:, ts(cb, 128)],
                                 x4[:, cb, w * 128:(w + 1) * 128],
                                 eye, start=(cb == 0),
                                 stop=(close and cb == 3),
                                 skip_group_check=True)
            return psO

        def pso_fin(psO, pscal, stage, wi, half=0):
            nc.tensor.matmul(psO[:], dupA if half == 0 else dupB,
                             pscal[:], start=False, stop=True,
                             skip_group_check=True)
            dst = stage[:, wi, :]
            if wi % 2 == 1:
                nc.vector.tensor_copy(dst, psO[:])
            else:
                nc.scalar.copy(dst, psO[:])

        hoist = [pso_mm(0, 0), pso_mm(0, 1)]
        pq = [p4_head(0, psL=psL0), p4_head(1)]
        for jc in range(32):
            pscal = pq.pop(0)
            stage = stagepool.tile([128, 2, 512], F16, tag="st",
                                   name="stage")
            stage2 = stagepool.tile([128, 2, 512], F16, tag="st",
                                    name="stage2")
            if jc == 0:
                ps0, ps1 = hoist
            else:
                ps0 = pso_mm(jc, 0)
                ps1 = pso_mm(jc, 1)
            pso_fin(ps0, pscal, stage, 0, half=0)
            if jc < 30:
                pq.append(p4_head(jc + 2))
            pso_fin(ps1, pscal, stage, 1, half=0)
            dview = d["out"][jc * 512:(jc + 1) * 512, :].rearrange(
                "(wi p) c -> p wi c", p=128)
            ps2 = pso_mm(jc, 2)
            nc.sync.dma_start(dview[:, 0:2, :], stage[:])
            pso_fin(ps2, pscal, stage2, 0, half=1)
            ps3 = pso_mm(jc, 3)
            pso_fin(ps3, pscal, stage2, 1, half=1)
            if jc < 31:
                nc.sync.dma_start(dview[:, 2:4, :], stage2[:])
            else:
                nc.sync.dma_start(dview[:, 2:3, :], stage2[:, 0:1, :])
                nc.sync.dma_start(dview[:, 3:4, :], stage2[:, 1:2, :])


# ------------------------------------------------------------------
# host-side wrapper
# ------------------------------------------------------------------
_NC_CACHE = None


def _get_program():
    global _NC_CACHE
    if _NC_CACHE is None:
        _NC_CACHE = build_program()
    return _NC_CACHE


def _make_in_map(xb, wq, bq, wk, bk, wv, bv, gamma):
    g = float(np.asarray(gamma).reshape(-1)[0])

    import ml_dtypes
    f8 = ml_dtypes.float8_e4m3
    cb16 = np.zeros((128, CB16_COLS), dtype=np.float16)
    cb8 = np.zeros((128, CB8_COLS), dtype=f8)
    wqT = (0.5 * np.asarray(wq, np.float32)).T    # (C, M)
    wkT = (0.5 * np.asarray(wk, np.float32)).T
    wvT = (0.5 * np.asarray(wv, np.float32)).T    # (C, C)
    for cb in range(4):
        rows = slice(cb * 128, (cb + 1) * 128)
        cb8[:, O8_WQ + cb * 64:O8_WQ + (cb + 1) * 64] = wqT[rows].astype(f8)
        cb8[:, O8_WK + cb * 64:O8_WK + (cb + 1) * 64] = wkT[rows].astype(f8)
        cb8[:, O8_WV + cb * 512:O8_WV + (cb + 1) * 512] = wvT[rows].astype(f8)
    cb16[:, O_BVB:O_BVB + 512] = np.asarray(bv, np.float32)[None, :]
    cb16[:, O_BKR:O_BKR + 64] = np.asarray(bk, np.float32)[None, :]
    ey = np.eye(128, dtype=np.float16)
    cb16[:, O_EYE:O_EYE + 128] = ey
    cb16[:, O_NEYE:O_NEYE + 128] = -0.25 * ey
    r = np.arange(128)
    dupA = np.zeros((128, 128), dtype=np.float16)
    dupA[r // 2, r] = 1.0
    dupB = np.zeros((128, 128), dtype=np.float16)
    dupB[64 + r // 2, r] = 1.0
    cb16[:, O_DUPA:O_DUPA + 128] = dupA
    cb16[:, O_DUPB:O_DUPB + 128] = dupB
    cb16[:, O_ONES:O_ONES + 128] = 1.0

    cb32 = np.zeros((128, 66), dtype=np.float32)
    cb32[0:M, 0] = np.asarray(bq, np.float32)
    cb32[:, 1] = 0.5 * g
    cb32[:, 2:66] = np.asarray(bk, np.float32)[None, :]

    return {
        "xb": np.ascontiguousarray(
            np.asarray(xb).reshape(C, N)).astype(np.float16),
        "cb16": cb16,
        "cb8": cb8,
        "cb32": cb32,
        "ones_row": np.ones((1, NL), dtype=np.float16),
    }


def kernel(x, y, gamma, gamma_y, wq, bq, wk, bk, wv, bv,
           wqy, bqy, wky, bky, wvy, bvy):
    x = np.asarray(x, dtype=np.float32)
    y = np.asarray(y, dtype=np.float32)
    B = x.shape[0]
    assert x.shape == (B, N, C), x.shape

    nc = _get_program()
    in_maps = []
    for b in range(B):
        in_maps.append(_make_in_map(x[b], wq, bq, wk, bk, wv, bv, gamma))
    for b in range(B):
        in_maps.append(_make_in_map(y[b], wqy, bqy, wky, bky, wvy, bvy,
                                    gamma_y))
    res = bass_utils.run_bass_kernel_spmd(
        nc, in_maps, core_ids=list(range(8)))
    out_x = np.stack([res.results[b]["out"].astype(np.float32)
                      for b in range(B)])
    out_y = np.stack([res.results[B + b]["out"].astype(np.float32)
                      for b in range(B)])
    return (out_x, out_y)
